# revision 1
# baseline (speedup 1.0000x reference)
"""DCRNN decoder (2-layer DCGRU, diffusion graph conv) on 8 trn2 cores.

Sharding: data-parallel over batch B=64 -> 8 batches/core; supports and
weights replicated. No collectives.

Per-core dataflow (all SBUF-resident after initial load):
  state kept in two layouts:
    natural  [node_part, feat]  -> stationary (lhsT) for aggregation matmuls
    transposed [feat_part, node] -> elementwise gate math + PE-transposes
  aggregation: aggrT[f, n_out] = sum_k feat[k, f] * S_T[m][k, n_out]
     (bf16 x bf16 -> fp32 PSUM, accumulated over 8 k-tiles)
  projection:  gate[gh, n_out] += W[m][f, gh]^T @ aggrT (fp32r, PSUM over m)
  gates: ACT sigmoid/tanh (+bias) -> bf16 transposed tiles
  update: h' = c + u*(h-c) on DVE; PE-transpose pairs of batches back to
  natural layout for the next aggregation.
"""

import sys

import numpy as np
import ml_dtypes

for _p in ("/opt/trn_rl_repo", "/root/.axon_site/_ro/trn_rl_repo"):
    if _p not in sys.path:
        sys.path.append(_p)

import concourse.bass as bass
import concourse.mybir as mybir
import concourse.tile as tile
from concourse.bass import ds
from concourse.bass_utils import run_bass_kernel_spmd

F32 = mybir.dt.float32
F32R = mybir.dt.float32r
BF16 = mybir.dt.bfloat16

NCORES = 8
BC = 8          # batches per core
N = 1000        # nodes
NPAD = 1024
KT = 8          # node (contraction) tiles of 128
NT = 8          # node output tiles of 128
H = 64
M = 4           # supports
NSTEP = 11      # time steps (T-1)
CH = 500        # n_out chunk (2 chunks of 500 per matmul free dim)
NPAIR = 4       # batch pairs

bf16 = ml_dtypes.bfloat16


def _nt_slice(nt):
    lo = 128 * nt
    hi = min(lo + 128, N)
    return lo, hi


def build_program(use_f32r_proj=True, dyn_loop=True, nstep=NSTEP, state_f32=True):
    nc = bass.Bass()
    SDT = F32 if state_f32 else BF16

    # ---- DRAM I/O ----
    st_d = nc.dram_tensor("st", [128, M, KT, N], BF16, kind="ExternalInput")
    h0n_d = nc.dram_tensor("h0n", [128, KT, BC, H], BF16, kind="ExternalInput")
    h1n_d = nc.dram_tensor("h1n", [128, KT, BC, H], BF16, kind="ExternalInput")
    h0t_d = nc.dram_tensor("h0t", [128, NPAIR, N], SDT, kind="ExternalInput")
    h1t_d = nc.dram_tensor("h1t", [128, NPAIR, N], SDT, kind="ExternalInput")
    xseq_d = nc.dram_tensor("xseq", [nstep, 128, KT, BC], BF16, kind="ExternalInput")
    w0ru_d = nc.dram_tensor("w0ru", [128, M, 128], F32R, kind="ExternalInput")
    w0c_d = nc.dram_tensor("w0c", [128, M, H], F32R, kind="ExternalInput")
    w1ru_d = nc.dram_tensor("w1ru", [128, M, 128], F32R, kind="ExternalInput")
    w1c_d = nc.dram_tensor("w1c", [128, M, H], F32R, kind="ExternalInput")
    bias_d = nc.dram_tensor("biases", [128, 6], F32, kind="ExternalInput")
    wproj_d = nc.dram_tensor("wproj", [128, 2], SDT, kind="ExternalInput")
    pbias_d = nc.dram_tensor("pbias", [2, 1], F32, kind="ExternalInput")
    identf_d = nc.dram_tensor("identf", [128, 128], SDT, kind="ExternalInput")
    identb_d = nc.dram_tensor("identb", [128, 128], BF16, kind="ExternalInput")
    out_d = nc.dram_tensor("out", [BC, nstep, N], F32, kind="ExternalOutput")

    pdt = F32R if use_f32r_proj else F32

    with tile.TileContext(nc) as tc:
        with (
            tc.tile_pool(name="const", bufs=1) as const,
            tc.tile_pool(name="agsb", bufs=4) as agsb_pool,
            tc.tile_pool(name="gates", bufs=4) as gate_pool,
            tc.tile_pool(name="upool", bufs=3) as u_pool,
            tc.tile_pool(name="hload", bufs=2) as hload_pool,
            tc.tile_pool(name="htmp", bufs=2) as htmp_pool,
            tc.tile_pool(name="outb", bufs=1) as out_pool,
            tc.tile_pool(name="ps_agg", bufs=3, space="PSUM") as ps_agg,
            tc.tile_pool(name="ps_gate", bufs=3, space="PSUM") as ps_gate,
            tc.tile_pool(name="ps_tp", bufs=2, space="PSUM") as ps_tp,
        ):
            # ---- resident tiles ----
            st = const.tile([128, M, KT, N], BF16, tag="st")
            feat0 = const.tile([128, KT, BC, 65], BF16, tag="feat0")
            feat1 = const.tile([128, KT, BC, 128], BF16, tag="feat1")
            h0t = const.tile([128, NPAIR, N], SDT, tag="h0t")
            h1t = const.tile([128, NPAIR, N], SDT, tag="h1t")
            rhT = const.tile([128, NPAIR, N], BF16, tag="rhT")
            w0ru = const.tile([128, M, 128], F32R, tag="w0ru")
            w0c = const.tile([128, M, H], F32R, tag="w0c")
            w1ru = const.tile([128, M, 128], F32R, tag="w1ru")
            w1c = const.tile([128, M, H], F32R, tag="w1c")
            biases = const.tile([128, 6], F32, tag="biases")
            wproj = const.tile([128, 2], SDT, tag="wproj")
            pbias = const.tile([2, 1], F32, tag="pbias")
            identf = const.tile([128, 128], SDT, tag="identf")
            identb = const.tile([128, 128], BF16, tag="identb")

            # ---- initial loads ----
            nc.vector.memset(feat0, 0.0)
            nc.vector.memset(feat1, 0.0)
            nc.sync.dma_start(out=st, in_=st_d[:])
            nc.sync.dma_start(out=feat0[:, :, :, 1:65], in_=h0n_d[:])
            nc.sync.dma_start(out=feat1[:, :, :, 64:128], in_=h1n_d[:])
            nc.sync.dma_start(out=h0t, in_=h0t_d[:])
            nc.sync.dma_start(out=h1t, in_=h1t_d[:])
            nc.sync.dma_start(out=w0ru, in_=w0ru_d[:])
            nc.sync.dma_start(out=w0c, in_=w0c_d[:])
            nc.sync.dma_start(out=w1ru, in_=w1ru_d[:])
            nc.sync.dma_start(out=w1c, in_=w1c_d[:])
            nc.sync.dma_start(out=biases, in_=bias_d[:])
            nc.sync.dma_start(out=wproj, in_=wproj_d[:])
            nc.sync.dma_start(out=pbias, in_=pbias_d[:])
            nc.sync.dma_start(out=identf, in_=identf_d[:])
            nc.sync.dma_start(out=identb, in_=identb_d[:])

            def aggr_block(b, feat, fwid, ch):
                """Aggregation for one (batch, chunk): 4 per-m aggrT tiles."""
                js = ds(ch * CH, CH)
                ags = []
                for m in range(M):
                    ag_ps = ps_agg.tile([128, CH], F32, tag="agg")
                    for kt in range(KT):
                        nc.tensor.matmul(
                            ag_ps[0:fwid, :],
                            lhsT=feat[:, kt, b, 0:fwid],
                            rhs=st[:, m, kt, js],
                            start=(kt == 0),
                            stop=(kt == KT - 1),
                        )
                    ag = agsb_pool.tile([128, CH], F32R, tag="agsb")
                    nc.vector.tensor_copy(ag[0:fwid, :], ag_ps[0:fwid, :])
                    ags.append(ag)
                return ags

            def proj_gate(ags, fwid, wtile, wcol, bcol, act_fn, out_sb, ch):
                """out_sb[0:64, ch] = act(sum_m W[m][:,wcol:wcol+64]^T @ aggrT + b)."""
                js = ds(ch * CH, CH)
                g_ps = ps_gate.tile([H, CH], F32, tag="gate")
                for m in range(M):
                    nc.tensor.matmul(
                        g_ps,
                        lhsT=wtile[0:fwid, m, wcol : wcol + H],
                        rhs=ags[m][0:fwid, :],
                        start=(m == 0),
                        stop=(m == M - 1),
                    )
                nc.scalar.activation(
                    out=out_sb[0:H, js],
                    in_=g_ps,
                    func=act_fn,
                    bias=biases[0:H, bcol : bcol + 1],
                )

            def transpose_to(src_tile, p, dests):
                """PE-transpose src_tile[:, p, :] (bf16 [128, N]) into natural
                layout; write [node, 2b*64f] to each dest (feat, col_lo)."""
                sdt = src_tile.dtype
                idt = identb if sdt == BF16 else identf
                for nt in range(NT):
                    lo, hi = _nt_slice(nt)
                    w = hi - lo
                    tp = ps_tp.tile([128, 128], sdt, tag="tp")
                    nc.tensor.transpose(
                        tp[0:w, :], src_tile[:, p, lo:hi], idt
                    )
                    for feat, col in dests:
                        nc.vector.tensor_copy(
                            feat[0:w, nt, 2 * p : 2 * p + 2, col : col + H],
                            tp[0:w, :].rearrange("p (b f) -> p b f", b=2),
                        )

            def layer(lidx, t_iv):
                feat = feat0 if lidx == 0 else feat1
                fwid = 65 if lidx == 0 else 128
                wru = w0ru if lidx == 0 else w1ru
                wc = w0c if lidx == 0 else w1c
                ht = h0t if lidx == 0 else h1t
                bcol = 3 * lidx
                rh_col = 1 if lidx == 0 else 64
                SIG = mybir.ActivationFunctionType.Sigmoid
                TANH = mybir.ActivationFunctionType.Tanh

                for p in range(NPAIR):
                    uT = {}
                    hsl = {}
                    for b in (2 * p, 2 * p + 1):
                        odd = b % 2
                        if odd:
                            hsrc = hload_pool.tile([H, N], SDT, tag="hload")
                            nc.sync.dma_start(out=hsrc, in_=ht[H:128, p, :])
                            hsl[b] = hsrc
                        else:
                            hsl[b] = ht[0:H, p, :]
                        rT = gate_pool.tile([H, N], SDT, tag="rT")
                        u_tile = u_pool.tile([H, N], SDT, tag="uT")
                        uT[b] = u_tile
                        for ch in range(2):
                            ags = aggr_block(b, feat, fwid, ch)
                            proj_gate(ags, fwid, wru, 0, bcol, SIG, rT, ch)
                            proj_gate(ags, fwid, wru, H, bcol + 1, SIG, u_tile, ch)
                        # rh = r * h  (transposed layout, base 0)
                        if odd:
                            rh_tmp = gate_pool.tile([H, N], BF16, tag="rT")
                            nc.vector.tensor_mul(rh_tmp, rT, hsl[b])
                            nc.sync.dma_start(out=rhT[H:128, p, :], in_=rh_tmp)
                        else:
                            nc.vector.tensor_mul(rhT[0:H, p, :], rT, hsl[b])
                    # rh -> natural (overwrites h cols of feat; h cols dead)
                    transpose_to(rhT, p, [(feat, rh_col)])
                    for b in (2 * p, 2 * p + 1):
                        odd = b % 2
                        cT = gate_pool.tile([H, N], SDT, tag="cT")
                        for ch in range(2):
                            ags = aggr_block(b, feat, fwid, ch)
                            proj_gate(ags, fwid, wc, 0, bcol + 2, TANH, cT, ch)
                        tmp = htmp_pool.tile([H, N], SDT, tag="htmp")
                        # h' = c + u*(h-c)
                        nc.vector.tensor_sub(tmp, hsl[b], cT)
                        nc.vector.tensor_mul(tmp, uT[b], tmp)
                        if odd:
                            hnew = htmp_pool.tile([H, N], SDT, tag="htmp")
                            nc.vector.tensor_add(hnew, cT, tmp)
                            nc.sync.dma_start(out=ht[H:128, p, :], in_=hnew)
                        else:
                            nc.vector.tensor_add(ht[0:H, p, :], cT, tmp)
                    if lidx == 0:
                        # h0' natural -> feat1 x-part and feat0 state cols
                        transpose_to(h0t, p, [(feat1, 0), (feat0, 1)])
                    else:
                        # h1' natural -> feat1 state cols
                        transpose_to(h1t, p, [(feat1, 64)])
                        # output projection for this pair
                        ob = out_pool.tile([2, N], F32, tag="outb")
                        for ch in range(2):
                            js = ds(ch * CH, CH)
                            o_ps = ps_tp.tile([2, CH], F32, tag="tp")
                            nc.tensor.matmul(
                                o_ps,
                                lhsT=wproj[:],
                                rhs=h1t[:, p, js],
                                start=True,
                                stop=True,
                            )
                            nc.scalar.activation(
                                out=ob[:, js],
                                in_=o_ps,
                                func=mybir.ActivationFunctionType.Identity,
                                bias=pbias[:],
                            )
                        nc.sync.dma_start(
                            out=out_d[2 * p : 2 * p + 2, ds(t_iv, 1), :].squeeze(1),
                            in_=ob,
                        )

            def step_body(t_iv):
                nc.sync.dma_start(
                    out=feat0[:, :, :, 0:1].squeeze(3),
                    in_=xseq_d[ds(t_iv, 1), :, :, :].squeeze(0),
                )
                layer(0, t_iv)
                layer(1, t_iv)

            if dyn_loop:
                with tc.For_i(0, nstep, 1, hint_engines=(mybir.EngineType.PE,)) as t:
                    step_body(t)
            else:
                for t in range(nstep):
                    step_body(t)

    _split_excess_waits(nc)
    return nc


def prep_inputs(inputs, state_f32=True):
    sdt = np.float32 if state_f32 else bf16
    """Host-side shard + relayout. Returns list of per-core in_maps."""
    S = np.asarray(inputs["supports"], np.float32)          # [M,N,N]
    ih = np.asarray(inputs["init_hidden"], np.float32)      # [2,B,N,H]
    x = np.asarray(inputs["input"], np.float32)[:, :, :, 0]  # [B,T,N]
    B = x.shape[0]

    # supports, transposed + padded: st[p,m,kt,j] = S[m][j,128kt+p]
    Sp = np.zeros((M, N, NPAD), np.float32)
    Sp[:, :, :N] = S
    st = Sp.reshape(M, N, KT, 128).transpose(3, 0, 2, 1).astype(bf16).copy()

    # weights
    f0 = 1 + H
    w0ru = np.zeros((128, M, 128), np.float32)
    w0c = np.zeros((128, M, H), np.float32)
    w1ru = np.zeros((128, M, 128), np.float32)
    w1c = np.zeros((128, M, H), np.float32)
    for m in range(M):
        w0ru[0:f0, m, 0:H] = inputs["w0_r"][m]
        w0ru[0:f0, m, H:128] = inputs["w0_u"][m]
        w0c[0:f0, m, :] = inputs["w0_c"][m]
        w1ru[:, m, 0:H] = inputs["w1_r"][m]
        w1ru[:, m, H:128] = inputs["w1_u"][m]
        w1c[:, m, :] = inputs["w1_c"][m]
    biases = np.zeros((128, 6), np.float32)
    for half in (0, 1):
        r0, r1 = half * H, half * H + H
        biases[r0:r1, 0] = inputs["b0_r"]
        biases[r0:r1, 1] = inputs["b0_u"]
        biases[r0:r1, 2] = inputs["b0_c"]
        biases[r0:r1, 3] = inputs["b1_r"]
        biases[r0:r1, 4] = inputs["b1_u"]
        biases[r0:r1, 5] = inputs["b1_c"]
    wproj = np.zeros((128, 2), np.float32)
    wproj[0:H, 0] = np.asarray(inputs["proj_w"], np.float32)[:, 0]
    wproj[H:128, 1] = np.asarray(inputs["proj_w"], np.float32)[:, 0]
    wproj = wproj.astype(sdt)
    pbias = np.full((2, 1), np.asarray(inputs["proj_b"], np.float32).reshape(()),
                    np.float32)
    identf = np.eye(128, dtype=sdt)
    identb = np.eye(128, dtype=bf16)

    common = dict(st=st, w0ru=w0ru, w0c=w0c, w1ru=w1ru, w1c=w1c,
                  biases=biases, wproj=wproj, pbias=pbias, identf=identf, identb=identb)

    in_maps = []
    for core in range(NCORES):
        bsl = slice(core * BC, (core + 1) * BC)
        ihc = ih[:, bsl]                                    # [2,8,N,H]
        ihp = np.zeros((2, BC, NPAD, H), np.float32)
        ihp[:, :, :N] = ihc
        hn = ihp.reshape(2, BC, KT, 128, H).transpose(0, 3, 2, 1, 4)  # [2,128,KT,BC,H]
        htr = ihc.transpose(0, 1, 3, 2).reshape(2, NPAIR, 2, H, N)
        htr = htr.transpose(0, 1, 2, 3, 4).reshape(2, NPAIR, 2 * H, N)
        htr = htr.transpose(0, 2, 1, 3)                     # [2,128,NPAIR,N]
        xc = x[bsl, :NSTEP]                                 # [8,11,N]
        xp = np.zeros((BC, NSTEP, NPAD), np.float32)
        xp[:, :, :N] = xc
        xseq = xp.reshape(BC, NSTEP, KT, 128).transpose(1, 3, 2, 0)  # [11,128,KT,BC]
        in_maps.append(dict(
            common,
            h0n=hn[0].astype(bf16).copy(),
            h1n=hn[1].astype(bf16).copy(),
            h0t=htr[0].astype(sdt).copy(),
            h1t=htr[1].astype(sdt).copy(),
            xseq=xseq.astype(bf16).copy(),
        ))
    return in_maps



_WAIT_CAP = 1


def _split_excess_waits(nc):
    """Walrus codegen here accepts at most 2 sync-wait commands per
    instruction; Tile can emit more.  Move excess waits onto injected
    same-engine no-ops placed immediately before the instruction."""
    for fn in nc.m.functions:
        for blk in fn.blocks:
            insts = list(blk.instructions)
            out = []
            for inst in insts:
                si = getattr(inst, "sync_info", None)
                waits = list(si.on_wait) if si and si.on_wait else []
                if len(waits) > _WAIT_CAP:
                    extra, keep = waits[:-_WAIT_CAP], waits[-_WAIT_CAP:]
                    while extra:
                        chunk, extra = extra[:_WAIT_CAP], extra[_WAIT_CAP:]
                        out.append(mybir.InstNoOp(
                            name=f"I-wsplit-{nc.next_id()}",
                            engine=inst.engine,
                            bass_nofuse=True,
                            sync_info=mybir.SyncInfo(on_wait=chunk, on_update=[]),
                        ))
                    si.on_wait = keep
                out.append(inst)
            if len(out) != len(insts):
                try:
                    blk.instructions = out
                except Exception:
                    blk.instructions.clear()
                    blk.instructions.extend(out)


_CACHE = {}


def _get_program(**kw):
    key = tuple(sorted(kw.items()))
    if key not in _CACHE:
        _CACHE[key] = build_program(**kw)
    return _CACHE[key]


def kernel(**inputs):
    nc = _get_program()
    in_maps = prep_inputs(inputs)
    res = run_bass_kernel_spmd(nc, in_maps, core_ids=list(range(NCORES)))
    outs = [res.results[c]["out"] for c in range(NCORES)]   # each [8,11,1000] f32
    full = np.concatenate(outs, axis=0)                     # [64,11,1000]
    return full[:, :, :, None].astype(np.float32)           # [B,T-1,N,1]


if __name__ == "__main__":
    nc = build_program()
    print("program built:", len(nc.m.functions[0].instructions) if hasattr(nc.m.functions[0], "instructions") else "ok")



# revision 12
# speedup vs baseline: 1.6437x; 1.6437x over previous
"""DCRNN decoder (2-layer DCGRU, diffusion graph conv) on 8 trn2 cores.

Sharding: data-parallel over batch B=64 -> 8 batches/core; supports and
weights replicated. No collectives.

Per-core dataflow (all SBUF-resident after initial load):
  state kept in two layouts:
    natural  [node_part, feat]  fp8, k-interleaved -> stationary (lhsT)
      for fp8 DoubleRow aggregation matmuls (contraction 256/matmul)
    transposed [feat_part, node] f32 -> elementwise gate math + PE-transposes
  aggregation: aggrT[f, n_out] = sum_k feat[k, f] * S_T[m][k, n_out]
     (fp8 x fp8 DoubleRow -> fp32 PSUM, accumulated over 4 k2-tiles;
      S pre-scaled by SSCALE=512 to sit in fp8e4m3 range)
  projection:  gate[ru, n_out] += W[m][f, 0:128]^T @ aggrT (bf16, W/SSCALE,
      PSUM over m; r and u share one 128-row matmul)
  gates: ACT sigmoid/tanh (+bias) from PSUM (partition-shifted for u)
  update: h' = c + u*(h-c) on DVE; PE-transpose pairs of batches back to
  natural fp8 layout for the next aggregation.
"""

import sys

import numpy as np
import ml_dtypes

for _p in ("/opt/trn_rl_repo", "/root/.axon_site/_ro/trn_rl_repo"):
    if _p not in sys.path:
        sys.path.append(_p)

import concourse.bass as bass
import concourse.mybir as mybir
import concourse.tile as tile
from concourse.bass import ds
from concourse.bass_utils import run_bass_kernel_spmd

F32 = mybir.dt.float32
BF16 = mybir.dt.bfloat16
FP8 = mybir.dt.float8e4
DR = mybir.MatmulPerfMode.DoubleRow

NCORES = 8
BC = 8          # batches per core
N = 1000        # nodes
NPAD = 1024
KT2 = 4         # node (contraction) tiles of 256 (2-interleaved 128)
NT = 8          # node output tiles of 128
H = 64
M = 4           # supports
NSTEP = 11      # time steps (T-1)
CH = 500        # n_out chunk (2 chunks of 500 per matmul free dim)
NPAIR = 4       # batch pairs
SSCALE = 512.0  # fp8 pre-scale for supports

bf16 = ml_dtypes.bfloat16
f8 = ml_dtypes.float8_e4m3fn


def _nt_slice(nt):
    lo = 128 * nt
    hi = min(lo + 128, N)
    return lo, hi


def build_program(dyn_loop=True, nstep=NSTEP):
    nc = bass.Bass()
    SDT = F32

    # ---- DRAM I/O ----
    st_d = nc.dram_tensor("st", [128, M, KT2, 2, N], FP8, kind="ExternalInput")
    h0n_d = nc.dram_tensor("h0n", [128, KT2, BC, 2, H], FP8, kind="ExternalInput")
    h1n_d = nc.dram_tensor("h1n", [128, KT2, BC, 2, H], FP8, kind="ExternalInput")
    h0t_d = nc.dram_tensor("h0t", [128, NPAIR, N], SDT, kind="ExternalInput")
    h1t_d = nc.dram_tensor("h1t", [128, NPAIR, N], SDT, kind="ExternalInput")
    xseq_d = nc.dram_tensor("xseq", [nstep, 128, KT2, BC, 2], FP8, kind="ExternalInput")
    w0ru_d = nc.dram_tensor("w0ru", [128, M, 128], BF16, kind="ExternalInput")
    w0c_d = nc.dram_tensor("w0c", [128, M, H], BF16, kind="ExternalInput")
    w1ru_d = nc.dram_tensor("w1ru", [128, M, 128], BF16, kind="ExternalInput")
    w1c_d = nc.dram_tensor("w1c", [128, M, H], BF16, kind="ExternalInput")
    bias_d = nc.dram_tensor("biases", [128, 6], F32, kind="ExternalInput")
    wproj_d = nc.dram_tensor("wproj", [128, 2], SDT, kind="ExternalInput")
    pbias_d = nc.dram_tensor("pbias", [2, 1], F32, kind="ExternalInput")
    identf_d = nc.dram_tensor("identf", [128, 128], SDT, kind="ExternalInput")
    out_d = nc.dram_tensor("out", [BC, nstep, N], F32, kind="ExternalOutput")

    with tile.TileContext(nc) as tc:
        with (
            tc.tile_pool(name="const", bufs=1) as const,
            tc.tile_pool(name="agsb", bufs=4) as agsb_pool,
            tc.tile_pool(name="gates", bufs=4) as gate_pool,
            tc.tile_pool(name="upool", bufs=3) as u_pool,
            tc.tile_pool(name="hload", bufs=2) as hload_pool,
            tc.tile_pool(name="htmp", bufs=2) as htmp_pool,
            tc.tile_pool(name="outb", bufs=1) as out_pool,
            tc.tile_pool(name="ps_agg", bufs=4, space="PSUM") as ps_agg,
            tc.tile_pool(name="ps_gate", bufs=2, space="PSUM") as ps_gate,
            tc.tile_pool(name="ps_tp", bufs=2, space="PSUM") as ps_tp,
        ):
            # ---- resident tiles ----
            st = const.tile([128, M, KT2, 2, N], FP8, tag="st")
            feat0 = const.tile([128, KT2, BC, 2, 2, 64], FP8, tag="feat0")
            feat1 = const.tile([128, KT2, BC, 2, 2, 64], FP8, tag="feat1")
            h0t = const.tile([128, NPAIR, N], SDT, tag="h0t")
            h1t = const.tile([128, NPAIR, N], SDT, tag="h1t")
            rhT = const.tile([128, NPAIR, N], SDT, tag="rhT")
            w0ru = const.tile([128, M, 128], BF16, tag="w0ru")
            w0c = const.tile([128, M, H], BF16, tag="w0c")
            w1ru = const.tile([128, M, 128], BF16, tag="w1ru")
            w1c = const.tile([128, M, H], BF16, tag="w1c")
            biases = const.tile([128, 6], F32, tag="biases")
            wproj = const.tile([128, 2], SDT, tag="wproj")
            pbias = const.tile([2, 1], F32, tag="pbias")
            identf = const.tile([128, 128], SDT, tag="identf")
            xcur = const.tile([128, KT2, BC, 2], FP8, tag="xcur")

            # ---- initial loads ----
            nc.vector.memset(feat0, 0.0)
            nc.vector.memset(feat1, 0.0)
            nc.sync.dma_start(out=st, in_=st_d[:])
            nc.sync.dma_start(out=feat0[:, :, :, 0, :, :], in_=h0n_d[:])
            nc.sync.dma_start(out=feat1[:, :, :, 1, :, :], in_=h1n_d[:])
            nc.sync.dma_start(out=h0t, in_=h0t_d[:])
            nc.sync.dma_start(out=h1t, in_=h1t_d[:])
            nc.sync.dma_start(out=w0ru, in_=w0ru_d[:])
            nc.sync.dma_start(out=w0c, in_=w0c_d[:])
            nc.sync.dma_start(out=w1ru, in_=w1ru_d[:])
            nc.sync.dma_start(out=w1c, in_=w1c_d[:])
            nc.sync.dma_start(out=biases, in_=bias_d[:])
            nc.sync.dma_start(out=wproj, in_=wproj_d[:])
            nc.sync.dma_start(out=pbias, in_=pbias_d[:])
            nc.sync.dma_start(out=identf, in_=identf_d[:])

            def aggr_block(b, feat, fwid, ch):
                """Aggregation for one (batch, chunk): 4 per-m aggrT tiles.
                fp8 DoubleRow: contraction 256 per matmul, 4 accumulations."""
                js = ds(ch * CH, CH)
                gw1 = fwid - 64          # 1 for L0 (x), 64 for L1
                ags = []
                for m in range(M):
                    ag0_full = ps_agg.tile([128, 512], F32, tag="agg")
                    ag1_full = ps_agg.tile([128, 512], F32, tag="agg")
                    for kt2 in range(KT2):
                        nc.tensor.matmul(
                            ag0_full[0:64, 0:CH],
                            lhsT=feat[:, kt2, b, 0, :, :],
                            rhs=st[:, m, kt2, :, js],
                            start=(kt2 == 0),
                            stop=(kt2 == KT2 - 1),
                            perf_mode=DR,
                        )
                    for kt2 in range(KT2):
                        nc.tensor.matmul(
                            ag1_full[0:gw1, 0:CH],
                            lhsT=feat[:, kt2, b, 1, :, 0:gw1],
                            rhs=st[:, m, kt2, :, js],
                            start=(kt2 == 0),
                            stop=(kt2 == KT2 - 1),
                            perf_mode=DR,
                        )
                    ag = agsb_pool.tile([128, CH], BF16, tag="agsb")
                    nc.vector.tensor_copy(ag[0:64, :], ag0_full[0:64, 0:CH])
                    nc.scalar.copy(ag[64:fwid, :], ag1_full[0:gw1, 0:CH])
                    ags.append(ag)
                return ags

            def proj_ru(ags, fwid, wtile, bcol, rT, uT, ch):
                """One 128-row matmul group for r|u, then two ACTs
                (u partition-shifted from PSUM rows 64:128 to base 0)."""
                js = ds(ch * CH, CH)
                SIG = mybir.ActivationFunctionType.Sigmoid
                g_full = ps_gate.tile([128, 512], F32, tag="gate")
                g_ps = g_full[:, 0:CH]
                for m in range(M):
                    nc.tensor.matmul(
                        g_ps,
                        lhsT=wtile[0:fwid, m, :],
                        rhs=ags[m][0:fwid, :],
                        start=(m == 0),
                        stop=(m == M - 1),
                    )
                nc.scalar.activation(
                    out=rT[:, js], in_=g_ps[0:H, :], func=SIG,
                    bias=biases[0:H, bcol : bcol + 1],
                )
                nc.scalar.activation(
                    out=uT[:, js], in_=g_ps[H:128, :], func=SIG,
                    bias=biases[H:128, bcol + 1 : bcol + 2],
                )

            def proj_c(ags, fwid, wtile, bcol, cT, ch):
                js = ds(ch * CH, CH)
                TANH = mybir.ActivationFunctionType.Tanh
                g_full = ps_gate.tile([H, 512], F32, tag="gate")
                g_ps = g_full[:, 0:CH]
                for m in range(M):
                    nc.tensor.matmul(
                        g_ps,
                        lhsT=wtile[0:fwid, m, :],
                        rhs=ags[m][0:fwid, :],
                        start=(m == 0),
                        stop=(m == M - 1),
                    )
                nc.scalar.activation(
                    out=cT[:, js], in_=g_ps, func=TANH,
                    bias=biases[0:H, bcol : bcol + 1],
                )

            def transpose_to(src_tile, p, dests):
                """PE-transpose src_tile[:, p, :] (f32 [128, N]) into natural
                fp8 k-interleaved layout; write [node, 2b*64f] to each dest
                (feat, group)."""
                for nt in range(NT):
                    lo, hi = _nt_slice(nt)
                    w = hi - lo
                    kt2, s = nt // 2, nt % 2
                    tp = ps_tp.tile([128, 128], SDT, tag="tp")
                    nc.tensor.transpose(
                        tp[0:w, :], src_tile[:, p, lo:hi], identf
                    )
                    for feat, grp in dests:
                        nc.vector.tensor_copy(
                            feat[0:w, kt2, 2 * p : 2 * p + 2, grp, s, :],
                            tp[0:w, :].rearrange("p (b f) -> p b f", b=2),
                        )

            def layer(lidx, t_iv):
                feat = feat0 if lidx == 0 else feat1
                fwid = 65 if lidx == 0 else 128
                wru = w0ru if lidx == 0 else w1ru
                wc = w0c if lidx == 0 else w1c
                ht = h0t if lidx == 0 else h1t
                bcol = 3 * lidx
                rh_grp = 0 if lidx == 0 else 1

                for p in range(NPAIR):
                    uT = {}
                    hsl = {}
                    for b in (2 * p, 2 * p + 1):
                        odd = b % 2
                        if odd:
                            hsrc = hload_pool.tile([H, N], SDT, tag="hload")
                            nc.sync.dma_start(out=hsrc, in_=ht[H:128, p, :])
                            hsl[b] = hsrc
                        else:
                            hsl[b] = ht[0:H, p, :]
                        rT = gate_pool.tile([H, N], SDT, tag="rT")
                        u_tile = u_pool.tile([H, N], SDT, tag="uT")
                        uT[b] = u_tile
                        for ch in range(2):
                            ags = aggr_block(b, feat, fwid, ch)
                            proj_ru(ags, fwid, wru, bcol, rT, u_tile, ch)
                        # rh = r * h  (transposed layout, base 0)
                        if odd:
                            rh_tmp = gate_pool.tile([H, N], SDT, tag="rT")
                            nc.vector.tensor_mul(rh_tmp, rT, hsl[b])
                            nc.sync.dma_start(out=rhT[H:128, p, :], in_=rh_tmp)
                        else:
                            nc.vector.tensor_mul(rhT[0:H, p, :], rT, hsl[b])
                    # rh -> natural (overwrites h cols of feat; h cols dead)
                    transpose_to(rhT, p, [(feat, rh_grp)])
                    for b in (2 * p, 2 * p + 1):
                        odd = b % 2
                        cT = gate_pool.tile([H, N], SDT, tag="cT")
                        for ch in range(2):
                            ags = aggr_block(b, feat, fwid, ch)
                            proj_c(ags, fwid, wc, bcol + 2, cT, ch)
                        tmp = htmp_pool.tile([H, N], SDT, tag="htmp")
                        # h' = c + u*(h-c)
                        nc.vector.tensor_sub(tmp, hsl[b], cT)
                        nc.vector.tensor_mul(tmp, uT[b], tmp)
                        if odd:
                            hnew = htmp_pool.tile([H, N], SDT, tag="htmp")
                            nc.vector.tensor_add(hnew, cT, tmp)
                            nc.sync.dma_start(out=ht[H:128, p, :], in_=hnew)
                        else:
                            nc.vector.tensor_add(ht[0:H, p, :], cT, tmp)
                    if lidx == 0:
                        # h0' natural -> feat1 g0 and feat0 g0
                        transpose_to(h0t, p, [(feat1, 0), (feat0, 0)])
                    else:
                        # h1' natural -> feat1 g1
                        transpose_to(h1t, p, [(feat1, 1)])
                        # output projection for this pair
                        ob = out_pool.tile([2, N], F32, tag="outb")
                        for ch in range(2):
                            js = ds(ch * CH, CH)
                            o_ps = ps_tp.tile([2, CH], F32, tag="tp")
                            nc.tensor.matmul(
                                o_ps,
                                lhsT=wproj[:],
                                rhs=h1t[:, p, js],
                                start=True,
                                stop=True,
                            )
                            nc.scalar.activation(
                                out=ob[:, js],
                                in_=o_ps,
                                func=mybir.ActivationFunctionType.Identity,
                                bias=pbias[:],
                            )
                        nc.sync.dma_start(
                            out=out_d[2 * p : 2 * p + 2, ds(t_iv, 1), :].squeeze(1),
                            in_=ob,
                        )

            def step_body(t_iv):
                nc.sync.dma_start(
                    out=xcur,
                    in_=xseq_d[ds(t_iv, 1), :, :, :, :].squeeze(0),
                )
                nc.vector.tensor_copy(
                    feat0[:, :, :, 1, :, 0:1], xcur[:, :, :, :, None],
                )
                layer(0, t_iv)
                layer(1, t_iv)

            if dyn_loop:
                with tc.For_i(0, nstep, 1, hint_engines=(mybir.EngineType.PE,)) as t:
                    step_body(t)
            else:
                for t in range(nstep):
                    step_body(t)

    _split_excess_waits(nc)
    return nc


def prep_inputs(inputs):
    """Host-side shard + relayout. Returns list of per-core in_maps."""
    S = np.asarray(inputs["supports"], np.float32)          # [M,N,N]
    ih = np.asarray(inputs["init_hidden"], np.float32)      # [2,B,N,H]
    x = np.asarray(inputs["input"], np.float32)[:, :, :, 0]  # [B,T,N]

    # supports, transposed + padded + k-interleaved + fp8-scaled:
    # st[p,m,kt2,s,j] = S[m][j, 256*kt2 + 128*s + p] * SSCALE
    Sp = np.zeros((M, N, NPAD), np.float32)
    Sp[:, :, :N] = S * SSCALE
    st = (Sp.reshape(M, N, KT2, 2, 128).transpose(4, 0, 2, 3, 1)
          .astype(f8).copy())

    # weights (merged r|u cols; divided by SSCALE to undo support scaling)
    f0 = 1 + H
    w0ru = np.zeros((128, M, 128), np.float32)
    w0c = np.zeros((128, M, H), np.float32)
    w1ru = np.zeros((128, M, 128), np.float32)
    w1c = np.zeros((128, M, H), np.float32)
    ord0 = list(range(1, f0)) + [0]   # h feats first, x last (row 64)
    for m in range(M):
        w0ru[0:f0, m, 0:H] = np.asarray(inputs["w0_r"], np.float32)[m][ord0]
        w0ru[0:f0, m, H:128] = np.asarray(inputs["w0_u"], np.float32)[m][ord0]
        w0c[0:f0, m, :] = np.asarray(inputs["w0_c"], np.float32)[m][ord0]
        w1ru[:, m, 0:H] = inputs["w1_r"][m]
        w1ru[:, m, H:128] = inputs["w1_u"][m]
        w1c[:, m, :] = inputs["w1_c"][m]
    w0ru = (w0ru / SSCALE).astype(bf16)
    w0c = (w0c / SSCALE).astype(bf16)
    w1ru = (w1ru / SSCALE).astype(bf16)
    w1c = (w1c / SSCALE).astype(bf16)
    biases = np.zeros((128, 6), np.float32)
    for half in (0, 1):
        r0, r1 = half * H, half * H + H
        biases[r0:r1, 0] = inputs["b0_r"]
        biases[r0:r1, 1] = inputs["b0_u"]
        biases[r0:r1, 2] = inputs["b0_c"]
        biases[r0:r1, 3] = inputs["b1_r"]
        biases[r0:r1, 4] = inputs["b1_u"]
        biases[r0:r1, 5] = inputs["b1_c"]
    wproj = np.zeros((128, 2), np.float32)
    wproj[0:H, 0] = np.asarray(inputs["proj_w"], np.float32)[:, 0]
    wproj[H:128, 1] = np.asarray(inputs["proj_w"], np.float32)[:, 0]
    pbias = np.full((2, 1), np.asarray(inputs["proj_b"], np.float32).reshape(()),
                    np.float32)
    identf = np.eye(128, dtype=np.float32)

    common = dict(st=st, w0ru=w0ru, w0c=w0c, w1ru=w1ru, w1c=w1c,
                  biases=biases, wproj=wproj, pbias=pbias, identf=identf)

    in_maps = []
    for core in range(NCORES):
        bsl = slice(core * BC, (core + 1) * BC)
        ihc = ih[:, bsl]                                    # [2,8,N,H]
        ihp = np.zeros((2, BC, NPAD, H), np.float32)
        ihp[:, :, :N] = ihc
        # [2,128,KT2,BC,2,H]: hn[l,p,kt2,b,s,f] = ih[l,b,256kt2+128s+p,f]
        hn = (ihp.reshape(2, BC, KT2, 2, 128, H)
              .transpose(0, 4, 2, 1, 3, 5))
        htr = ihc.transpose(0, 1, 3, 2).reshape(2, NPAIR, 2, H, N)
        htr = htr.reshape(2, NPAIR, 2 * H, N)
        htr = htr.transpose(0, 2, 1, 3)                     # [2,128,NPAIR,N]
        xc = x[bsl, :NSTEP]                                 # [8,11,N]
        xp = np.zeros((BC, NSTEP, NPAD), np.float32)
        xp[:, :, :N] = xc
        # [NSTEP,128,KT2,BC,2]
        xseq = (xp.reshape(BC, NSTEP, KT2, 2, 128)
                .transpose(1, 4, 2, 0, 3))
        in_maps.append(dict(
            common,
            h0n=hn[0].astype(f8).copy(),
            h1n=hn[1].astype(f8).copy(),
            h0t=htr[0].astype(np.float32).copy(),
            h1t=htr[1].astype(np.float32).copy(),
            xseq=xseq.astype(f8).copy(),
        ))
    return in_maps


_WAIT_CAP = 1


def _split_excess_waits(nc):
    """Walrus codegen here accepts at most 2 sync-wait commands per
    instruction; Tile can emit more.  Move excess waits onto injected
    same-engine no-ops placed immediately before the instruction."""
    for fn in nc.m.functions:
        for blk in fn.blocks:
            insts = list(blk.instructions)
            out = []
            for inst in insts:
                si = getattr(inst, "sync_info", None)
                waits = list(si.on_wait) if si and si.on_wait else []
                if len(waits) > _WAIT_CAP:
                    extra, keep = waits[:-_WAIT_CAP], waits[-_WAIT_CAP:]
                    while extra:
                        chunk, extra = extra[:_WAIT_CAP], extra[_WAIT_CAP:]
                        out.append(mybir.InstNoOp(
                            name=f"I-wsplit-{nc.next_id()}",
                            engine=inst.engine,
                            bass_nofuse=True,
                            sync_info=mybir.SyncInfo(on_wait=chunk, on_update=[]),
                        ))
                    si.on_wait = keep
                out.append(inst)
            if len(out) != len(insts):
                try:
                    blk.instructions = out
                except Exception:
                    blk.instructions.clear()
                    blk.instructions.extend(out)


_CACHE = {}


def _get_program(**kw):
    key = tuple(sorted(kw.items()))
    if key not in _CACHE:
        _CACHE[key] = build_program(**kw)
    return _CACHE[key]


def kernel(**inputs):
    nc = _get_program()
    in_maps = prep_inputs(inputs)
    res = run_bass_kernel_spmd(nc, in_maps, core_ids=list(range(NCORES)))
    outs = [res.results[c]["out"] for c in range(NCORES)]   # each [8,11,1000] f32
    full = np.concatenate(outs, axis=0)                     # [64,11,1000]
    return full[:, :, :, None].astype(np.float32)           # [B,T-1,N,1]


if __name__ == "__main__":
    nc = build_program()
    print("program built ok")
